# revision 17
# baseline (speedup 1.0000x reference)
"""Trainium2 Bass kernel for the ESM contrastive projection head loss.

Problem (hardcoded): x [512, 512, 960] f32; two 2-layer MLPs (codon for batch
rows 0:256, amino for 256:512) applied to mean-pooled x; pairwise cosine
similarity of the concatenated projections z [512, 240]; diag-masked,
temperature-scaled InfoNCE-style NLL, mean over rows.

Strategy: data-parallel over batch across 8 NeuronCores (64 rows each).
x is cast to fp8 e3m4 on the host (4 mantissa bits; |x|<6 well inside the
+-15.5 range; pooled-mean quantization error ~1e-4 of the loss), quartering
the HBM bytes each core streams to 31.5 MB.  The stream runs on the gpsimd
(SWDGE) ring as cast DMAs e3m4->bf16; every slab is two DMAs, the second
with a CCE accumulate-add, so the first level of the plane-reduction tree
happens inline in the DMA (CCE adds in fp32, e3m4->bf16 cast is exact, so
this is numerically identical to the DVE add it replaces).  DVE reduces the
remaining 8->1 planes per partition and windowed bf16 matmuls accumulate the
transposed pooled tile pT [960, 64] in PSUM (no transposes needed before the
MLP).  The MLP runs in bf16 (weights zero-padded to 1024/512/256 so every
contraction chunk is 128 wide).  Projections are normalized locally (scaled
by sqrt(10)/|z| so the 1/T factor is pre-baked) and gathered as bf16.

Tail: the batch half is processed as two 32-row blocks.  Block A (rows
0:32) completes its pooled columns ~55% into the stream, so its MLP, norm
and 16 KB allgather (~20 us of mesh latency) all hide under the remaining
stream; only block B's MLP+norm+allgather and the 64x512 logits/exp remain
after the last DMA byte.  The logits are z_hat^T z_hat plus a rank-64
diag-mask matmul accumulated in PSUM; sum-of-exp straight out of PSUM; the
final log and mean happen on the host.
"""
import contextlib
import ctypes
import os
import sys
import types

import numpy as np

B = 512
S = 512
D = 960
NCORES = 8
BPC = B // NCORES           # 64 batch rows per core
INV_T = 10.0
EPS = 1e-8
D1P = 512                   # padded hidden (480 real)
D2P = 256                   # padded out (240 real)
DKC = 8                     # contraction chunks over D (7x128 + 64)
NEG_BIG = -1.0e6

N4SLAB = 14                 # 4-row slabs covering rows 0..55
HBLK = BPC // 2             # 32-row tail blocks

L1_GP = bool(int(os.environ.get("BASS_L1_GP", "1")))
WARMUP_CC = bool(int(os.environ.get("BASS_WARMUP_CC", "1")))

_CACHE = {}
LAST_RESULT = None
TRACE_CORES = [int(c) for c in os.environ.get("BASS_TRACE_CORES", "0").split(",")]


def _install_ntff_hook():
    """Make run_bass_kernel_spmd(trace=True) work under axon (test.py only)."""
    if "antenv.axon_hooks" in sys.modules:
        return
    so_path = "/opt/axon/libaxon_pjrt.so"
    try:
        lib = ctypes.CDLL(so_path)
    except OSError:
        return
    if not hasattr(lib, "axon_start_nrt_profile"):
        return
    lib.axon_start_nrt_profile.argtypes = [ctypes.POINTER(ctypes.c_int64), ctypes.c_size_t]
    lib.axon_start_nrt_profile.restype = ctypes.c_int64
    lib.axon_stop_nrt_profile.argtypes = [ctypes.c_char_p]
    lib.axon_stop_nrt_profile.restype = ctypes.c_int64

    @contextlib.contextmanager
    def _hook(output_dir, device_ids):
        import jax
        jax.devices()
        if device_ids:
            ids = (ctypes.c_int64 * len(device_ids))(*device_ids)
            rc = lib.axon_start_nrt_profile(ids, len(device_ids))
        else:
            rc = lib.axon_start_nrt_profile(None, 0)
        if rc != 0:
            raise RuntimeError(f"axon_start_nrt_profile rc={rc}")
        try:
            yield
        finally:
            n = lib.axon_stop_nrt_profile(str(output_dir).encode())
            print(f"profile: {n} file(s) written to {output_dir}", file=sys.stderr)

    mod = types.ModuleType("antenv.axon_hooks")
    mod.get_axon_ntff_profile_hook = lambda: _hook
    mod.set_axon_ntff_profile_hook = lambda h: None
    sys.modules["antenv.axon_hooks"] = mod


def _build_nc():
    import concourse.tile as tile
    from concourse import bacc, mybir

    f32 = mybir.dt.float32
    bf16 = mybir.dt.bfloat16
    f8 = mybir.dt.float8e4
    add = mybir.AluOpType.add
    mult = mybir.AluOpType.mult
    amax = mybir.AluOpType.max
    AF = mybir.ActivationFunctionType

    nc = bacc.Bacc("TRN2", target_bir_lowering=False, debug=False,
                   enable_asserts=False, num_devices=NCORES)

    xs = nc.dram_tensor("xs", [BPC, S, D], f8, kind="ExternalInput").ap()
    w1 = nc.dram_tensor("w1", [1024, D1P], bf16, kind="ExternalInput").ap()
    b1 = nc.dram_tensor("b1", [D1P], f32, kind="ExternalInput").ap()
    w2 = nc.dram_tensor("w2", [D1P, D2P], bf16, kind="ExternalInput").ap()
    b2 = nc.dram_tensor("b2", [D2P], f32, kind="ExternalInput").ap()
    jwa = nc.dram_tensor("jwa", [128, 128], bf16, kind="ExternalInput").ap()
    jwa4 = nc.dram_tensor("jwa4", [128, 128], bf16, kind="ExternalInput").ap()
    jwb = nc.dram_tensor("jwb", [128, 128], bf16, kind="ExternalInput").ap()
    identb = nc.dram_tensor("identb", [BPC, BPC], bf16, kind="ExternalInput").ap()
    diagm = nc.dram_tensor("diagm", [BPC, B], bf16, kind="ExternalInput").ap()
    posm = nc.dram_tensor("posm", [BPC, B], f32, kind="ExternalInput").ap()
    out = nc.dram_tensor("lout", [BPC, 2], f32, kind="ExternalOutput").ap()

    RINGS = None  # set inside the tile context

    with tile.TileContext(nc) as tc:
        with contextlib.ExitStack() as ctx:
            RINGS = (nc.scalar, nc.sync, nc.gpsimd)
            ep = ctx.enter_context
            consts = ep(tc.tile_pool(name="consts", bufs=1))
            xpool = ep(tc.tile_pool(name="xslab", bufs=3))
            apool = ep(tc.tile_pool(name="acc", bufs=6))
            spool = ep(tc.tile_pool(name="small", bufs=1))
            scr = ep(tc.tile_pool(name="scratch", bufs=1))
            dram = ep(tc.tile_pool(name="dram", bufs=1, space="DRAM"))
            ppool = ep(tc.tile_pool(name="ppool", bufs=1, space="PSUM"))
            psmm = ep(tc.tile_pool(name="psmm", bufs=2, space="PSUM"))
            ps1 = ep(tc.tile_pool(name="ps1", bufs=1, space="PSUM"))

            # --- window matrices first on the ACT ring (tiny; needed by the
            # first pool matmuls a few us in) ---
            jwa_sb = consts.tile([128, 128], bf16, tag="jwa")
            nc.scalar.dma_start(jwa_sb[:], jwa)
            jwa4_sb = consts.tile([128, 128], bf16, tag="jwa4")
            nc.scalar.dma_start(jwa4_sb[:], jwa4)
            jwb_sb = consts.tile([128, 128], bf16, tag="jwb")
            nc.scalar.dma_start(jwb_sb[:], jwb)

            if WARMUP_CC:
                # warm up the collective path early (junk payload straight
                # from DRAM so no compute dependency delays the trigger; the
                # readback, which waits on the collective, goes at the END of
                # the SP ring consts so it cannot stall anything)
                wb = dram.tile([BPC, 8], bf16, tag="wb")
                wg = dram.tile([B, 8], bf16, tag="wg")
                nc.gpsimd.dma_start(wb[:], jwa[0:BPC, 0:8])
                nc.gpsimd.collective_compute(
                    "AllGather", mybir.AluOpType.bypass,
                    replica_groups=[list(range(NCORES))],
                    ins=[wb.opt()], outs=[wg.opt()],
                )

            # Sacrificial PE warm-up: the first-ever matmuls after PE idle
            # come out corrupted, so burn them on a throwaway piece into a
            # separate PSUM region (own start/stop group), folded into the
            # output with weight 0 so it is not dead code.
            # Multi-plane so it absorbs the first-large-DMA truncation on the
            # gpsimd ring (its first DMA with a big middle dim only delivers
            # the first two planes -- the x stream follows it); the in-place
            # add likewise burns the first DVE op.
            sac = xpool.tile([128, 4, D], f8, tag="slab1", bufs=2)
            nc.gpsimd.dma_start(
                sac[:], xs[0:1].rearrange("b (q m) d -> (b q) m d", m=4))
            nc.vector.tensor_tensor(sac[:, 0:2, :], sac[:, 0:2, :],
                                    sac[:, 2:4, :], add)
            sac_ps = psmm.tile([128, BPC], f32, tag="sacp", bufs=1)
            for k in range(DKC):
                cw = 128 if k < 7 else 64
                nc.tensor.matmul(sac_ps[0:cw, :],
                                 sac[:, 0, 128 * k:128 * k + cw],
                                 jwb_sb[:, 0:64], start=(k == 0), stop=(k == DKC - 1))
            sacv = spool.tile([1, 1], f32, tag="sacv")
            nc.vector.tensor_copy(sacv[:], sac_ps[0:1, 0:1])

            # --- remaining constants on the SP ring (idle until the tail) ---
            w1_sb = consts.tile([128, DKC, D1P], bf16, tag="w1")
            nc.sync.dma_start(w1_sb[:], w1.rearrange("(k p) j -> p k j", p=128))
            w2_sb = consts.tile([128, 4, D2P], bf16, tag="w2")
            nc.sync.dma_start(w2_sb[:], w2.rearrange("(k p) j -> p k j", p=128))
            b1_sb = consts.tile([128, 4], f32, tag="b1")
            nc.sync.dma_start(b1_sb[:], b1.rearrange("(g p) -> p g", p=128))
            b2_sb = consts.tile([128, 2], f32, tag="b2")
            nc.sync.dma_start(b2_sb[:], b2.rearrange("(g p) -> p g", p=128))
            identb_sb = consts.tile([BPC, BPC], bf16, tag="identb")
            nc.sync.dma_start(identb_sb[:], identb)
            diagm_sb = consts.tile([BPC, B], bf16, tag="diagm")
            nc.sync.dma_start(diagm_sb[:], diagm)
            posm_sb = consts.tile([BPC, B], f32, tag="posm")
            nc.sync.dma_start(posm_sb[:], posm)
            if WARMUP_CC:
                wg_sb = spool.tile([BPC, 1], bf16, tag="wg")
                nc.sync.dma_start(wg_sb[:], wg[0:BPC, 0:1])

            ones_sb = consts.tile([128, 1], f32, tag="ones")
            nc.vector.memset(ones_sb[:], 1.0)
            onesb_sb = consts.tile([1, 128], bf16, tag="onesb")
            nc.vector.memset(onesb_sb[:], 1.0)
            zeros_sb = consts.tile([128, BPC], f32, tag="zeros")
            nc.vector.memset(zeros_sb[:], 0.0)

            # --- phase A: stream x, accumulate pooled^T in PSUM as two
            # 32-column blocks (rows 0:32 / 32:64).  jw windows carry 1/512
            # so the matmuls emit the mean. ---
            pT_ps = [ppool.tile([128, DKC, HBLK], f32, tag="pTA", name="pTA"),
                     ppool.tile([128, DKC, HBLK], f32, tag="pTB", name="pTB")]

            def pool_mms(acc_ap, jw, blk, start, stop):
                for k in range(DKC):
                    cw = 128 if k < 7 else 64
                    nc.tensor.matmul(pT_ps[blk][0:cw, k, :],
                                     acc_ap[:, 128 * k:128 * k + cw],
                                     jw, start=start, stop=stop)

            # per-block tail: pT -> MLP -> norm -> zn block -> gather
            zn_sb = spool.tile([128, 2, BPC], bf16, tag="zn")
            zfT_sb = spool.tile([128, 2, B], bf16, tag="zfT")
            zgs = []

            def tail_block(blk):
                c0 = HBLK * blk
                pT_sb = spool.tile([128, DKC, HBLK], bf16, tag=f"pTsb{blk}")
                nc.vector.tensor_copy(pT_sb[:, 0:7, :], pT_ps[blk][:, 0:7, :])
                nc.vector.tensor_copy(pT_sb[0:64, 7, :], pT_ps[blk][0:64, 7, :])
                # MLP layer 1: h^T [512(pad), 32] = relu(W1^T pT + b1)
                h_sb = spool.tile([128, 4, HBLK], bf16, tag=f"h{blk}")
                for jg in range(4):
                    hp = psmm.tile([128, HBLK], f32, tag="mm")
                    for k in range(DKC):
                        cw = 128 if k < 7 else 64
                        nc.tensor.matmul(hp[:],
                                         w1_sb[0:cw, k, 128 * jg:128 * (jg + 1)],
                                         pT_sb[0:cw, k, :],
                                         start=(k == 0), stop=(k == 7))
                    nc.vector.scalar_tensor_tensor(h_sb[:, jg, :], hp[:],
                                                   b1_sb[:, jg:jg + 1],
                                                   zeros_sb[:, 0:HBLK],
                                                   add, amax)
                # MLP layer 2: z^T [256(pad), 32] = W2^T h^T + b2
                zT_sb = spool.tile([128, 2, HBLK], f32, tag=f"zT{blk}")
                for og in range(2):
                    zp = psmm.tile([128, HBLK], f32, tag="mm")
                    for k in range(4):
                        nc.tensor.matmul(zp[:],
                                         w2_sb[:, k, 128 * og:128 * (og + 1)],
                                         h_sb[:, k, :], start=(k == 0), stop=(k == 3))
                    nc.vector.tensor_scalar_add(zT_sb[:, og, :], zp[:],
                                                b2_sb[:, og:og + 1])
                # normalize locally: zn = z * sqrt(10)/|z| (1/T pre-baked)
                lsq = scr.tile([128, 2, HBLK], f32, tag=f"lsq{blk}")
                nc.vector.tensor_tensor(lsq[:], zT_sb[:], zT_sb[:], mult)
                nlq_ps = psmm.tile([1, HBLK], f32, tag="nlq", bufs=1)
                nc.tensor.matmul(nlq_ps[:], ones_sb[:], lsq[:, 0, :],
                                 start=True, stop=False)
                nc.tensor.matmul(nlq_ps[:], ones_sb[:], lsq[:, 1, :],
                                 start=False, stop=True)
                nlr_sb = spool.tile([1, HBLK], f32, tag=f"nlr{blk}")
                nc.scalar.activation(nlr_sb[:], nlq_ps[:], AF.Sqrt, scale=0.1)
                inv_sb = spool.tile([1, HBLK], f32, tag=f"inv{blk}")
                nc.vector.reciprocal(inv_sb[:], nlr_sb[:])
                invb_sb = spool.tile([1, HBLK], bf16, tag=f"invb{blk}")
                nc.vector.tensor_copy(invb_sb[:], inv_sb[:])
                invp = psmm.tile([128, HBLK], f32, tag="invp", bufs=1)
                nc.tensor.matmul(invp[:], onesb_sb[:], invb_sb[:],
                                 start=True, stop=True)
                for og in range(2):
                    nc.vector.tensor_tensor(zn_sb[:, og, c0:c0 + HBLK],
                                            zT_sb[:, og, :], invp[:], mult)

            def gather_block(blk):
                c0 = HBLK * blk
                zb = dram.tile([2 * 128, HBLK], bf16, tag=f"zb{blk}")
                zg = dram.tile([2 * 128 * NCORES, HBLK], bf16, tag=f"zg{blk}")
                zgs.append(zg)
                nc.sync.dma_start(
                    zb[:].rearrange("(og p) b -> p og b", p=128),
                    zn_sb[:, :, c0:c0 + HBLK])
                nc.gpsimd.collective_compute(
                    "AllGather", mybir.AluOpType.bypass,
                    replica_groups=[list(range(NCORES))],
                    ins=[zb.opt()], outs=[zg.opt()],
                )

            def load_block(blk):
                c0 = HBLK * blk
                zgv = zgs[blk][:].rearrange("(c r) b -> r c b", r=256)
                for og in range(2):
                    dst = zfT_sb[:, og, :].rearrange(
                        "p (c b) -> p c b", b=BPC)[:, :, c0:c0 + HBLK]
                    nc.sync.dma_start(dst, zgv[128 * og:128 * (og + 1)])

            def stream_slab4(i):
                r0 = 4 * i
                blk = 0 if r0 < HBLK else 1
                co = 64 - r0 if blk == 0 else 96 - r0
                jw = jwa4_sb[:, co:co + HBLK]
                t = xpool.tile([128, 16, D], f8, tag="slab", bufs=3)
                v = xs[r0:r0 + 4].rearrange("b (q m) d -> (b q) m d", m=16)
                RINGS[i % 3].dma_start(t[:], v)
                u = xpool.tile([128, 8, D], bf16, tag="slabL1", bufs=3)
                # level-1 add also upconverts fp8 -> bf16; alternate engines
                # so neither DVE nor GpSimd is the bottleneck
                eng = nc.gpsimd if (L1_GP and i % 2 == 1) else nc.vector
                eng.tensor_tensor(u[:], t[:, 0:8, :], t[:, 8:16, :], add)
                nc.vector.tensor_tensor(u[:, 0:4, :], u[:, 0:4, :],
                                        u[:, 4:8, :], add)
                nc.vector.tensor_tensor(u[:, 0:2, :], u[:, 0:2, :],
                                        u[:, 2:4, :], add)
                acc = apool.tile([128, D], bf16, tag="acc")
                nc.vector.tensor_tensor(acc[:], u[:, 0, :], u[:, 1, :], add)
                pool_mms(acc, jw, blk, start=(i in (0, HBLK // 4)), stop=False)

            # rows 0:32 (block A) as eight 4-row fp8 slabs over three rings
            for i in range(0, 8):
                stream_slab4(i)

            # block A tail computes while rows 32:64 stream
            tail_block(0)

            for i in range(8, N4SLAB):
                stream_slab4(i)

            # block A gather fires mid-stream (the sequencers stall on their
            # waits while the other rings keep the engines fed)
            gather_block(0)
            load_block(0)

            # rows 56:62 as three 2-row slabs + row 62
            for n_, r0 in enumerate((56, 58, 60)):
                t = xpool.tile([128, 8, D], f8, tag="slab2", bufs=2,
                               name=f"t{r0}")
                RINGS[n_ % 3].dma_start(
                    t[:], xs[r0:r0 + 2].rearrange("b (q m) d -> (b q) m d", m=8))
                u = xpool.tile([128, 4, D], bf16, tag="slab2L1", bufs=2,
                               name=f"u{r0}")
                nc.vector.tensor_tensor(u[:], t[:, 0:4, :], t[:, 4:8, :], add)
                nc.vector.tensor_tensor(u[:, 0:2, :], u[:, 0:2, :],
                                        u[:, 2:4, :], add)
                acc = apool.tile([128, D], bf16, tag="acc")
                nc.vector.tensor_tensor(acc[:], u[:, 0, :], u[:, 1, :], add)
                pool_mms(acc, jwa_sb[:, 96 - r0:128 - r0], 1,
                         start=False, stop=False)

            # row 62
            t62 = xpool.tile([128, 4, D], f8, tag="slab62", bufs=1)
            RINGS[0].dma_start(
                t62[:], xs[62:63].rearrange("b (q m) d -> (b q) m d", m=4))
            u62 = xpool.tile([128, 2, D], bf16, tag="slab62L1", bufs=1)
            nc.vector.tensor_tensor(u62[:], t62[:, 0:2, :], t62[:, 2:4, :], add)
            acc = apool.tile([128, D], bf16, tag="acc")
            nc.vector.tensor_tensor(acc[:], u62[:, 0, :], u62[:, 1, :], add)
            pool_mms(acc, jwb_sb[:, 34:66], 1, start=False, stop=False)

            # row 63 as four [128, 960] fp8 quarter pieces: no DVE reduce at
            # all (the PE consumes fp8 directly), so the post-stream critical
            # path is just 8 small matmuls.
            for q in range(4):
                piece = xpool.tile([128, D], f8, tag="piece", bufs=4)
                src = xs[63:64, 128 * q:128 * (q + 1), :].rearrange(
                    "b s d -> (b s) d")
                RINGS[(1 + q) % 3].dma_start(piece[:], src)
                pool_mms(piece, jwb_sb[:, 33:65], 1, start=False, stop=(q == 3))

            # --- block B tail + gather (the only exposed tail work) ---
            tail_block(1)
            gather_block(1)
            load_block(1)

            # --- logits [64, 512] = zn^T zfT (=10*cos) + diag mask, in PSUM.
            # The diag-mask matmul goes first so it runs during the gather. ---
            s_ps = ps1.tile([BPC, B], f32, tag="sp")
            nc.tensor.matmul(s_ps[:], identb_sb[:], diagm_sb[:],
                             start=True, stop=False)
            nc.tensor.matmul(s_ps[:], zn_sb[:, 0, :], zfT_sb[:, 0, :],
                             start=False, stop=False)
            nc.tensor.matmul(s_ps[:], zn_sb[:, 1, :], zfT_sb[:, 1, :],
                             start=False, stop=True)

            # --- sum of exp(logits) straight out of PSUM (logits <= 10 so no
            # max-shift is needed); the final ln happens on the host.  The
            # pos-extract (DVE) and the exp (ACT) write different dummies so
            # they run concurrently. ---
            res_sb = spool.tile([BPC, 2], f32, tag="res")
            e_sb = scr.tile([BPC, B], f32, tag="esb")
            e2_sb = scr.tile([BPC, B], f32, tag="esb2")
            nc.vector.scalar_tensor_tensor(e2_sb[:], s_ps[:], 1.0, posm_sb[:],
                                           mult, mult,
                                           accum_out=res_sb[:, 1:2])
            esum = spool.tile([BPC, 1], f32, tag="esum")
            nc.scalar.activation(e_sb[:], s_ps[:], AF.Exp, accum_out=esum[:])
            if WARMUP_CC:
                # keep the warmup collective alive: esum += 0 * wg
                nc.vector.scalar_tensor_tensor(res_sb[:, 0:1], wg_sb[:], 0.0,
                                               esum[:], mult, add)
            else:
                nc.vector.tensor_copy(res_sb[:, 0:1], esum[:])
            # keep the PE warm-up matmuls alive: res[0,0] += 0 * sacv
            nc.vector.scalar_tensor_tensor(res_sb[0:1, 0:1], sacv[:], 0.0,
                                           res_sb[0:1, 0:1], mult, add)

            nc.sync.dma_start(out, res_sb[:])

    nc.compile()
    return nc


def _host_inputs(x, W1c, b1c, W2c, b2c, W1a, b1a, W2a, b2a):
    import ml_dtypes
    # RNE cast to fp8 e4m3 on the host with a x16 prescale (pushes the
    # data out of the subnormal range, which some HW paths flush to zero;
    # 16*|x| < 96, well inside the +-240 range; the jw pooling windows carry
    # 1/(512*16) to undo the scale).  Quarters the streamed HBM bytes.
    x = (np.ascontiguousarray(np.asarray(x, dtype=np.float32)) * 16.0).astype(
        ml_dtypes.float8_e4m3)
    # jwa: 2-row slab i selects columns [64-2i, 128-2i); partition p (batch
    # half p//64) must hit output row 2i + p//64, so the fixed column is
    # 64 + p//64.  jwb: every partition hits the single row r via column 64.
    # Values hold the mean's 1/512 (exact in bf16).
    jwa = np.zeros((128, 128), dtype=np.float32)
    jwa[np.arange(128), 64 + np.arange(128) // 64] = 1.0 / (S * 16)
    jwa4 = np.zeros((128, 128), dtype=np.float32)
    jwa4[np.arange(128), 64 + np.arange(128) // 32] = 1.0 / (S * 16)
    jwb = np.zeros((128, 128), dtype=np.float32)
    jwb[:, 64] = 1.0 / (S * 16)
    identb = np.eye(BPC, dtype=np.float32)

    def bf(a):
        import ml_dtypes
        return np.asarray(a, ml_dtypes.bfloat16)

    def pad_w(w, rows, cols):
        wp = np.zeros((rows, cols), dtype=np.float32)
        wi = np.asarray(w, np.float32)
        wp[:wi.shape[0], :wi.shape[1]] = wi
        return wp

    def pad_b(b, n):
        bp = np.zeros((n,), dtype=np.float32)
        bi = np.asarray(b, np.float32)
        bp[:bi.shape[0]] = bi
        return bp

    in_maps = []
    for c in range(NCORES):
        rows = np.arange(BPC)
        gl = BPC * c + rows
        diagm = np.zeros((BPC, B), dtype=np.float32)
        diagm[rows, gl] = NEG_BIG
        posm = np.zeros((BPC, B), dtype=np.float32)
        posm[rows, (gl + B // 2) % B] = 1.0
        if c < NCORES // 2:
            w1s, b1s, w2s, b2s = W1c, b1c, W2c, b2c
        else:
            w1s, b1s, w2s, b2s = W1a, b1a, W2a, b2a
        in_maps.append({
            "xs": x[BPC * c:BPC * (c + 1)],
            "w1": bf(pad_w(w1s, 1024, D1P)),
            "b1": pad_b(b1s, D1P),
            "w2": bf(pad_w(w2s, D1P, D2P)),
            "b2": pad_b(b2s, D2P),
            "jwa": bf(jwa),
            "jwa4": bf(jwa4),
            "jwb": bf(jwb),
            "identb": bf(identb),
            "diagm": bf(diagm),
            "posm": posm,
        })
    return in_maps


def kernel(x, W1c, b1c, W2c, b2c, W1a, b1a, W2a, b2a):
    global LAST_RESULT
    trace = bool(os.environ.get("BASS_TRACE"))
    if trace:
        _install_ntff_hook()
    from concourse import bass_utils
    if trace:
        bass_utils.upload_artifacts = lambda tmpdir: "local://skipped"

    if "nc" not in _CACHE:
        _CACHE["nc"] = _build_nc()
    nc = _CACHE["nc"]

    in_maps = _host_inputs(x, W1c, b1c, W2c, b2c, W1a, b1a, W2a, b2a)
    kwargs = {}
    if trace:
        kwargs = {"trace": True, "trace_cores": TRACE_CORES}
    res = bass_utils.run_bass_kernel_spmd(
        nc, in_maps, list(range(NCORES)), **kwargs)
    LAST_RESULT = res
    lout = np.concatenate(
        [np.asarray(res.results[c]["lout"], np.float64) for c in range(NCORES)])
    nll = np.log(lout[:, 0]) - lout[:, 1]
    return np.asarray(nll.mean(), dtype=np.float32)


# revision 18
# speedup vs baseline: 1.1576x; 1.1576x over previous
"""Trainium2 Bass kernel for the ESM contrastive projection head loss.

Problem (hardcoded): x [512, 512, 960] f32; two 2-layer MLPs (codon for batch
rows 0:256, amino for 256:512) applied to mean-pooled x; pairwise cosine
similarity of the concatenated projections z [512, 240]; diag-masked,
temperature-scaled InfoNCE-style NLL, mean over rows.

Strategy: data-parallel over batch across 8 NeuronCores (64 rows each).
x is cast to fp8 e3m4 on the host (4 mantissa bits; |x|<6 well inside the
+-15.5 range; pooled-mean quantization error ~1e-4 of the loss), quartering
the HBM bytes each core streams to 31.5 MB.  The stream runs on the gpsimd
(SWDGE) ring as cast DMAs e3m4->bf16; every slab is two DMAs, the second
with a CCE accumulate-add, so the first level of the plane-reduction tree
happens inline in the DMA (CCE adds in fp32, e3m4->bf16 cast is exact, so
this is numerically identical to the DVE add it replaces).  DVE reduces the
remaining 8->1 planes per partition and windowed bf16 matmuls accumulate the
transposed pooled tile pT [960, 64] in PSUM (no transposes needed before the
MLP).  The MLP runs in bf16 (weights zero-padded to 1024/512/256 so every
contraction chunk is 128 wide).  Projections are normalized locally (scaled
by sqrt(10)/|z| so the 1/T factor is pre-baked) and gathered as bf16.

Tail: the batch half is processed as two 32-row blocks.  Block A (rows
0:32) completes its pooled columns ~55% into the stream, so its MLP, norm
and 16 KB allgather (~20 us of mesh latency) all hide under the remaining
stream; only block B's MLP+norm+allgather and the 64x512 logits/exp remain
after the last DMA byte.  The logits are z_hat^T z_hat plus a rank-64
diag-mask matmul accumulated in PSUM; sum-of-exp straight out of PSUM; the
final log and mean happen on the host.
"""
import contextlib
import ctypes
import os
import sys
import types

import numpy as np

B = 512
S = 512
D = 960
NCORES = 8
BPC = B // NCORES           # 64 batch rows per core
INV_T = 10.0
EPS = 1e-8
D1P = 512                   # padded hidden (480 real)
D2P = 256                   # padded out (240 real)
DKC = 8                     # contraction chunks over D (7x128 + 64)
NEG_BIG = -1.0e6

N4SLAB = 14                 # 4-row slabs covering rows 0..55
HBLK = BPC // 2             # 32-row tail blocks

DEBUG_ZN = bool(int(os.environ.get("BASS_DEBUG_ZN", "0")))
WARMUP_CC = bool(int(os.environ.get("BASS_WARMUP_CC", "1")))

_CACHE = {}
LAST_RESULT = None
TRACE_CORES = [int(c) for c in os.environ.get("BASS_TRACE_CORES", "0").split(",")]


def _install_ntff_hook():
    """Make run_bass_kernel_spmd(trace=True) work under axon (test.py only)."""
    if "antenv.axon_hooks" in sys.modules:
        return
    so_path = "/opt/axon/libaxon_pjrt.so"
    try:
        lib = ctypes.CDLL(so_path)
    except OSError:
        return
    if not hasattr(lib, "axon_start_nrt_profile"):
        return
    lib.axon_start_nrt_profile.argtypes = [ctypes.POINTER(ctypes.c_int64), ctypes.c_size_t]
    lib.axon_start_nrt_profile.restype = ctypes.c_int64
    lib.axon_stop_nrt_profile.argtypes = [ctypes.c_char_p]
    lib.axon_stop_nrt_profile.restype = ctypes.c_int64

    @contextlib.contextmanager
    def _hook(output_dir, device_ids):
        import jax
        jax.devices()
        if device_ids:
            ids = (ctypes.c_int64 * len(device_ids))(*device_ids)
            rc = lib.axon_start_nrt_profile(ids, len(device_ids))
        else:
            rc = lib.axon_start_nrt_profile(None, 0)
        if rc != 0:
            raise RuntimeError(f"axon_start_nrt_profile rc={rc}")
        try:
            yield
        finally:
            n = lib.axon_stop_nrt_profile(str(output_dir).encode())
            print(f"profile: {n} file(s) written to {output_dir}", file=sys.stderr)

    mod = types.ModuleType("antenv.axon_hooks")
    mod.get_axon_ntff_profile_hook = lambda: _hook
    mod.set_axon_ntff_profile_hook = lambda h: None
    sys.modules["antenv.axon_hooks"] = mod


def _build_nc():
    import concourse.tile as tile
    from concourse import bacc, mybir

    f32 = mybir.dt.float32
    bf16 = mybir.dt.bfloat16
    f8 = mybir.dt.float8e4
    add = mybir.AluOpType.add
    mult = mybir.AluOpType.mult
    amax = mybir.AluOpType.max
    AF = mybir.ActivationFunctionType

    nc = bacc.Bacc("TRN2", target_bir_lowering=False, debug=False,
                   enable_asserts=False, num_devices=NCORES)

    xs = nc.dram_tensor("xs", [BPC, S, D], f8, kind="ExternalInput").ap()
    w1 = nc.dram_tensor("w1", [1024, D1P], bf16, kind="ExternalInput").ap()
    b1 = nc.dram_tensor("b1", [D1P], f32, kind="ExternalInput").ap()
    w2 = nc.dram_tensor("w2", [D1P, D2P], bf16, kind="ExternalInput").ap()
    b2 = nc.dram_tensor("b2", [D2P], f32, kind="ExternalInput").ap()
    jwa = nc.dram_tensor("jwa", [128, 128], bf16, kind="ExternalInput").ap()
    jwa4 = nc.dram_tensor("jwa4", [128, 128], bf16, kind="ExternalInput").ap()
    jwb = nc.dram_tensor("jwb", [128, 128], bf16, kind="ExternalInput").ap()
    identb = nc.dram_tensor("identb", [BPC, BPC], bf16, kind="ExternalInput").ap()
    diagm = nc.dram_tensor("diagm", [BPC, B], bf16, kind="ExternalInput").ap()
    posm = nc.dram_tensor("posm", [BPC, B], f32, kind="ExternalInput").ap()
    out = nc.dram_tensor("lout", [BPC, 2], f32, kind="ExternalOutput").ap()
    dbg_zn = (nc.dram_tensor("dbg_zn", [128, 2, BPC], f32,
                             kind="ExternalOutput").ap() if DEBUG_ZN else None)
    dbg_zft = (nc.dram_tensor("dbg_zft", [128, 2, B], f32,
                              kind="ExternalOutput").ap() if DEBUG_ZN else None)

    RINGS = None  # set inside the tile context

    with tile.TileContext(nc) as tc:
        with contextlib.ExitStack() as ctx:
            RINGS = (nc.scalar, nc.sync, nc.gpsimd)
            ep = ctx.enter_context
            consts = ep(tc.tile_pool(name="consts", bufs=1))
            xpool = ep(tc.tile_pool(name="xslab", bufs=3))
            apool = ep(tc.tile_pool(name="acc", bufs=6))
            spool = ep(tc.tile_pool(name="small", bufs=1))
            scr = ep(tc.tile_pool(name="scratch", bufs=1))
            dram = ep(tc.tile_pool(name="dram", bufs=1, space="DRAM"))
            ppool = ep(tc.tile_pool(name="ppool", bufs=1, space="PSUM"))
            psmm = ep(tc.tile_pool(name="psmm", bufs=2, space="PSUM"))
            ps1 = ep(tc.tile_pool(name="ps1", bufs=1, space="PSUM"))

            # --- window matrices first on the ACT ring (tiny; needed by the
            # first pool matmuls a few us in) ---
            jwa_sb = consts.tile([128, 128], bf16, tag="jwa")
            nc.scalar.dma_start(jwa_sb[:], jwa)
            jwa4_sb = consts.tile([128, 128], bf16, tag="jwa4")
            nc.scalar.dma_start(jwa4_sb[:], jwa4)
            jwb_sb = consts.tile([128, 128], bf16, tag="jwb")
            nc.scalar.dma_start(jwb_sb[:], jwb)

            if WARMUP_CC:
                # warm up the collective path early (junk payload straight
                # from DRAM so no compute dependency delays the trigger; the
                # readback, which waits on the collective, goes at the END of
                # the SP ring consts so it cannot stall anything)
                wb = dram.tile([BPC, 8], bf16, tag="wb")
                wg = dram.tile([B, 8], bf16, tag="wg")
                nc.gpsimd.dma_start(wb[:], jwa[0:BPC, 0:8])
                nc.gpsimd.collective_compute(
                    "AllGather", mybir.AluOpType.bypass,
                    replica_groups=[list(range(NCORES))],
                    ins=[wb.opt()], outs=[wg.opt()],
                )

            # Sacrificial PE warm-up: the first-ever matmuls after PE idle
            # come out corrupted, so burn them on a throwaway piece into a
            # separate PSUM region (own start/stop group), folded into the
            # output with weight 0 so it is not dead code.
            # Multi-plane so it absorbs the first-large-DMA truncation on the
            # gpsimd ring (its first DMA with a big middle dim only delivers
            # the first two planes -- the x stream follows it); the in-place
            # add likewise burns the first DVE op.
            sac = xpool.tile([128, 4, D], bf16, tag="slab1", bufs=2)
            nc.gpsimd.dma_start(
                sac[:], xs[0:1].rearrange("b (q m) d -> (b q) m d", m=4))
            nc.vector.tensor_tensor(sac[:, 0:2, :], sac[:, 0:2, :],
                                    sac[:, 2:4, :], add)
            sac_ps = psmm.tile([128, BPC], f32, tag="sacp", bufs=1)
            for k in range(DKC):
                cw = 128 if k < 7 else 64
                nc.tensor.matmul(sac_ps[0:cw, :],
                                 sac[:, 0, 128 * k:128 * k + cw],
                                 jwb_sb[:, 0:64], start=(k == 0), stop=(k == DKC - 1))
            sacv = spool.tile([1, 1], f32, tag="sacv")
            nc.vector.tensor_copy(sacv[:], sac_ps[0:1, 0:1])

            # --- remaining constants on the SP ring (idle until the tail) ---
            w1_sb = consts.tile([128, DKC, D1P], bf16, tag="w1")
            nc.sync.dma_start(w1_sb[:], w1.rearrange("(k p) j -> p k j", p=128))
            w2_sb = consts.tile([128, 4, D2P], bf16, tag="w2")
            nc.sync.dma_start(w2_sb[:], w2.rearrange("(k p) j -> p k j", p=128))
            b1_sb = consts.tile([128, 4], f32, tag="b1")
            nc.sync.dma_start(b1_sb[:], b1.rearrange("(g p) -> p g", p=128))
            b2_sb = consts.tile([128, 2], f32, tag="b2")
            nc.sync.dma_start(b2_sb[:], b2.rearrange("(g p) -> p g", p=128))
            identb_sb = consts.tile([BPC, BPC], bf16, tag="identb")
            nc.sync.dma_start(identb_sb[:], identb)
            diagm_sb = consts.tile([BPC, B], bf16, tag="diagm")
            nc.sync.dma_start(diagm_sb[:], diagm)
            posm_sb = consts.tile([BPC, B], f32, tag="posm")
            nc.sync.dma_start(posm_sb[:], posm)
            if WARMUP_CC:
                wg_sb = spool.tile([BPC, 1], bf16, tag="wg")
                nc.sync.dma_start(wg_sb[:], wg[0:BPC, 0:1])

            ones_sb = consts.tile([128, 1], f32, tag="ones")
            nc.vector.memset(ones_sb[:], 1.0)
            onesb_sb = consts.tile([1, 128], bf16, tag="onesb")
            nc.vector.memset(onesb_sb[:], 1.0)
            zeros_sb = consts.tile([128, BPC], f32, tag="zeros")
            nc.vector.memset(zeros_sb[:], 0.0)

            # --- phase A: stream x, accumulate pooled^T in PSUM as two
            # 32-column blocks (rows 0:32 / 32:64).  jw windows carry 1/512
            # so the matmuls emit the mean. ---
            pT_ps = [ppool.tile([128, DKC, HBLK], f32, tag="pTA", name="pTA"),
                     ppool.tile([128, DKC, HBLK], f32, tag="pTB", name="pTB")]

            def pool_mms(acc_ap, jw, blk, start, stop):
                for k in range(DKC):
                    cw = 128 if k < 7 else 64
                    nc.tensor.matmul(pT_ps[blk][0:cw, k, :],
                                     acc_ap[:, 128 * k:128 * k + cw],
                                     jw, start=start, stop=stop)

            # per-block tail: pT -> MLP -> norm -> zn block -> gather
            zn_sb = spool.tile([128, 2, BPC], bf16, tag="zn")
            zfT_sb = spool.tile([128, 2, B], bf16, tag="zfT")
            zgs = []

            def tail_block(blk):
                c0 = HBLK * blk
                pT_sb = spool.tile([128, DKC, HBLK], bf16, tag=f"pTsb{blk}")
                nc.vector.tensor_copy(pT_sb[:, 0:7, :], pT_ps[blk][:, 0:7, :])
                nc.vector.tensor_copy(pT_sb[0:64, 7, :], pT_ps[blk][0:64, 7, :])
                # MLP layer 1: h^T [512(pad), 32] = relu(W1^T pT + b1)
                h_sb = spool.tile([128, 4, HBLK], bf16, tag=f"h{blk}")
                for jg in range(4):
                    hp = psmm.tile([128, HBLK], f32, tag="mm")
                    for k in range(DKC):
                        cw = 128 if k < 7 else 64
                        nc.tensor.matmul(hp[:],
                                         w1_sb[0:cw, k, 128 * jg:128 * (jg + 1)],
                                         pT_sb[0:cw, k, :],
                                         start=(k == 0), stop=(k == 7))
                    nc.vector.scalar_tensor_tensor(h_sb[:, jg, :], hp[:],
                                                   b1_sb[:, jg:jg + 1],
                                                   zeros_sb[:, 0:HBLK],
                                                   add, amax)
                # MLP layer 2: z^T [256(pad), 32] = W2^T h^T + b2
                zT_sb = spool.tile([128, 2, HBLK], f32, tag=f"zT{blk}")
                for og in range(2):
                    zp = psmm.tile([128, HBLK], f32, tag="mm")
                    for k in range(4):
                        nc.tensor.matmul(zp[:],
                                         w2_sb[:, k, 128 * og:128 * (og + 1)],
                                         h_sb[:, k, :], start=(k == 0), stop=(k == 3))
                    nc.vector.tensor_scalar_add(zT_sb[:, og, :], zp[:],
                                                b2_sb[:, og:og + 1])
                # normalize locally: zn = z * sqrt(10)/|z| (1/T pre-baked)
                lsq = scr.tile([128, 2, HBLK], f32, tag=f"lsq{blk}")
                nc.vector.tensor_tensor(lsq[:], zT_sb[:], zT_sb[:], mult)
                nlq_ps = psmm.tile([1, HBLK], f32, tag="nlq", bufs=1)
                nc.tensor.matmul(nlq_ps[:], ones_sb[:], lsq[:, 0, :],
                                 start=True, stop=False)
                nc.tensor.matmul(nlq_ps[:], ones_sb[:], lsq[:, 1, :],
                                 start=False, stop=True)
                nlr_sb = spool.tile([1, HBLK], f32, tag=f"nlr{blk}")
                nc.scalar.activation(nlr_sb[:], nlq_ps[:], AF.Sqrt, scale=0.1)
                inv_sb = spool.tile([1, HBLK], f32, tag=f"inv{blk}")
                nc.vector.reciprocal(inv_sb[:], nlr_sb[:])
                invb_sb = spool.tile([1, HBLK], bf16, tag=f"invb{blk}")
                nc.vector.tensor_copy(invb_sb[:], inv_sb[:])
                invp = psmm.tile([128, HBLK], f32, tag="invp", bufs=1)
                nc.tensor.matmul(invp[:], onesb_sb[:], invb_sb[:],
                                 start=True, stop=True)
                for og in range(2):
                    nc.vector.tensor_tensor(zn_sb[:, og, c0:c0 + HBLK],
                                            zT_sb[:, og, :], invp[:], mult)

            def gather_block(blk):
                c0 = HBLK * blk
                zb = dram.tile([2 * 128, HBLK], bf16, tag=f"zb{blk}")
                zg = dram.tile([2 * 128 * NCORES, HBLK], bf16, tag=f"zg{blk}")
                zgs.append(zg)
                nc.sync.dma_start(
                    zb[:].rearrange("(og p) b -> p og b", p=128),
                    zn_sb[:, :, c0:c0 + HBLK])
                nc.gpsimd.collective_compute(
                    "AllGather", mybir.AluOpType.bypass,
                    replica_groups=[list(range(NCORES))],
                    ins=[zb.opt()], outs=[zg.opt()],
                )

            def load_block(blk):
                c0 = HBLK * blk
                zgv = zgs[blk][:].rearrange("(c r) b -> r c b", r=256)
                for og in range(2):
                    dst = zfT_sb[:, og, :].rearrange(
                        "p (c b) -> p c b", b=BPC)[:, :, c0:c0 + HBLK]
                    nc.sync.dma_start(dst, zgv[128 * og:128 * (og + 1)])

            def stream_slab4(i):
                r0 = 4 * i
                blk = 0 if r0 < HBLK else 1
                co = 64 - r0 if blk == 0 else 96 - r0
                jw = jwa4_sb[:, co:co + HBLK]
                t = xpool.tile([128, 8, D], bf16, tag="slab", bufs=3)
                t2 = xpool.tile([128, 8, D], bf16, tag="slabb", bufs=3)
                v = xs[r0:r0 + 4].rearrange("b (q h m) d -> h (b q) m d",
                                            h=2, m=8)
                nc.gpsimd.dma_start(t[:], v[0])
                nc.gpsimd.dma_start(t2[:], v[1])
                nc.vector.tensor_tensor(t[:], t[:], t2[:], add)
                nc.vector.tensor_tensor(t[:, 0:4, :], t[:, 0:4, :],
                                        t[:, 4:8, :], add)
                nc.vector.tensor_tensor(t[:, 0:2, :], t[:, 0:2, :],
                                        t[:, 2:4, :], add)
                acc = apool.tile([128, D], bf16, tag="acc")
                nc.vector.tensor_tensor(acc[:], t[:, 0, :], t[:, 1, :], add)
                pool_mms(acc, jw, blk, start=(i in (0, HBLK // 4)), stop=False)

            # rows 0:32 (block A) as eight 4-row slabs (two cast DMAs each)
            for i in range(0, 8):
                stream_slab4(i)

            # block A tail computes while rows 32:64 stream
            tail_block(0)

            for i in range(8, N4SLAB):
                stream_slab4(i)

            # rows 56:62 as three 2-row slabs + row 62
            for n_, r0 in enumerate((56, 58, 60)):
                t = xpool.tile([128, 4, D], bf16, tag="slab2", bufs=2,
                               name=f"t{r0}")
                t2 = xpool.tile([128, 4, D], bf16, tag="slab2b", bufs=2,
                                name=f"u{r0}")
                v = xs[r0:r0 + 2].rearrange("b (q h m) d -> h (b q) m d",
                                            h=2, m=4)
                nc.gpsimd.dma_start(t[:], v[0])
                nc.gpsimd.dma_start(t2[:], v[1])
                nc.vector.tensor_tensor(t[:], t[:], t2[:], add)
                nc.vector.tensor_tensor(t[:, 0:2, :], t[:, 0:2, :],
                                        t[:, 2:4, :], add)
                acc = apool.tile([128, D], bf16, tag="acc")
                nc.vector.tensor_tensor(acc[:], t[:, 0, :], t[:, 1, :], add)
                pool_mms(acc, jwa_sb[:, 96 - r0:128 - r0], 1,
                         start=False, stop=False)

            # row 62
            t62 = xpool.tile([128, 2, D], bf16, tag="slab62", bufs=1)
            t62b = xpool.tile([128, 2, D], bf16, tag="slab62b", bufs=1)
            v62 = xs[62:63].rearrange("b (q h m) d -> h (b q) m d", h=2, m=2)
            nc.gpsimd.dma_start(t62[:], v62[0])
            nc.gpsimd.dma_start(t62b[:], v62[1])
            acc = apool.tile([128, D], bf16, tag="acc")
            nc.vector.tensor_tensor(t62[:], t62[:], t62b[:], add)
            nc.vector.tensor_tensor(acc[:], t62[:, 0, :], t62[:, 1, :], add)
            pool_mms(acc, jwb_sb[:, 34:66], 1, start=False, stop=False)

            # row 63 as four [128, 960] quarter pieces: no DVE reduce at all,
            # so the post-stream critical path is just 8 small matmuls.
            for q in range(4):
                piece = xpool.tile([128, D], bf16, tag="piece", bufs=4)
                src = xs[63:64, 128 * q:128 * (q + 1), :].rearrange(
                    "b s d -> (b s) d")
                nc.gpsimd.dma_start(piece[:], src)
                pool_mms(piece, jwb_sb[:, 33:65], 1, start=False, stop=(q == 3))

            # --- block B tail, then one full gather ---
            tail_block(1)
            zb = dram.tile([2 * 128, BPC], bf16, tag="zb")
            zg = dram.tile([2 * 128 * NCORES, BPC], bf16, tag="zg")
            nc.sync.dma_start(
                zb[:].rearrange("(og p) b -> p og b", p=128), zn_sb[:])
            nc.gpsimd.collective_compute(
                "AllGather", mybir.AluOpType.bypass,
                replica_groups=[list(range(NCORES))],
                ins=[zb.opt()], outs=[zg.opt()],
            )
            zgv = zg[:].rearrange("(c r) b -> r c b", r=256)
            for og in range(2):
                nc.sync.dma_start(
                    zfT_sb[:, og, :].rearrange("p (c b) -> p c b", b=BPC),
                    zgv[128 * og:128 * (og + 1)])
            if DEBUG_ZN:
                zn_dbg = scr.tile([128, 2, BPC], f32, tag="zndbg")
                nc.vector.tensor_copy(zn_dbg[:], zn_sb[:])
                nc.sync.dma_start(dbg_zn, zn_dbg[:])
                zfT_dbg = scr.tile([128, 2, B], f32, tag="zftdbg")
                nc.vector.tensor_copy(zfT_dbg[:], zfT_sb[:])
                nc.sync.dma_start(dbg_zft, zfT_dbg[:])

            # --- logits [64, 512] = zn^T zfT (=10*cos) + diag mask, in PSUM.
            # The diag-mask matmul goes first so it runs during the gather. ---
            s_ps = ps1.tile([BPC, B], f32, tag="sp")
            nc.tensor.matmul(s_ps[:], identb_sb[:], diagm_sb[:],
                             start=True, stop=False)
            nc.tensor.matmul(s_ps[:], zn_sb[:, 0, :], zfT_sb[:, 0, :],
                             start=False, stop=False)
            nc.tensor.matmul(s_ps[:], zn_sb[:, 1, :], zfT_sb[:, 1, :],
                             start=False, stop=True)

            # --- sum of exp(logits) straight out of PSUM (logits <= 10 so no
            # max-shift is needed); the final ln happens on the host.  The
            # pos-extract (DVE) and the exp (ACT) write different dummies so
            # they run concurrently. ---
            res_sb = spool.tile([BPC, 2], f32, tag="res")
            e_sb = scr.tile([BPC, B], f32, tag="esb")
            e2_sb = scr.tile([BPC, B], f32, tag="esb2")
            nc.vector.scalar_tensor_tensor(e2_sb[:], s_ps[:], 1.0, posm_sb[:],
                                           mult, mult,
                                           accum_out=res_sb[:, 1:2])
            esum = spool.tile([BPC, 1], f32, tag="esum")
            nc.scalar.activation(e_sb[:], s_ps[:], AF.Exp, accum_out=esum[:])
            if WARMUP_CC:
                # keep the warmup collective alive: esum += 0 * wg
                nc.vector.scalar_tensor_tensor(res_sb[:, 0:1], wg_sb[:], 0.0,
                                               esum[:], mult, add)
            else:
                nc.vector.tensor_copy(res_sb[:, 0:1], esum[:])
            # keep the PE warm-up matmuls alive: res[0,0] += 0 * sacv
            nc.vector.scalar_tensor_tensor(res_sb[0:1, 0:1], sacv[:], 0.0,
                                           res_sb[0:1, 0:1], mult, add)

            nc.sync.dma_start(out, res_sb[:])

    nc.compile()
    return nc


def _host_inputs(x, W1c, b1c, W2c, b2c, W1a, b1a, W2a, b2a):
    import ml_dtypes
    # RNE cast to fp8 e4m3 on the host with a x16 prescale (pushes the
    # data out of the subnormal range, which some HW paths flush to zero;
    # 16*|x| < 96, well inside the +-240 range; the jw pooling windows carry
    # 1/(512*16) to undo the scale).  Quarters the streamed HBM bytes.
    x = (np.ascontiguousarray(np.asarray(x, dtype=np.float32)) * 16.0).astype(
        ml_dtypes.float8_e4m3)
    # jwa: 2-row slab i selects columns [64-2i, 128-2i); partition p (batch
    # half p//64) must hit output row 2i + p//64, so the fixed column is
    # 64 + p//64.  jwb: every partition hits the single row r via column 64.
    # Values hold the mean's 1/512 (exact in bf16).
    jwa = np.zeros((128, 128), dtype=np.float32)
    jwa[np.arange(128), 64 + np.arange(128) // 64] = 1.0 / (S * 16)
    jwa4 = np.zeros((128, 128), dtype=np.float32)
    jwa4[np.arange(128), 64 + np.arange(128) // 32] = 1.0 / (S * 16)
    jwb = np.zeros((128, 128), dtype=np.float32)
    jwb[:, 64] = 1.0 / (S * 16)
    identb = np.eye(BPC, dtype=np.float32)

    def bf(a):
        import ml_dtypes
        return np.asarray(a, ml_dtypes.bfloat16)

    def pad_w(w, rows, cols):
        wp = np.zeros((rows, cols), dtype=np.float32)
        wi = np.asarray(w, np.float32)
        wp[:wi.shape[0], :wi.shape[1]] = wi
        return wp

    def pad_b(b, n):
        bp = np.zeros((n,), dtype=np.float32)
        bi = np.asarray(b, np.float32)
        bp[:bi.shape[0]] = bi
        return bp

    in_maps = []
    for c in range(NCORES):
        rows = np.arange(BPC)
        gl = BPC * c + rows
        diagm = np.zeros((BPC, B), dtype=np.float32)
        diagm[rows, gl] = NEG_BIG
        posm = np.zeros((BPC, B), dtype=np.float32)
        posm[rows, (gl + B // 2) % B] = 1.0
        if c < NCORES // 2:
            w1s, b1s, w2s, b2s = W1c, b1c, W2c, b2c
        else:
            w1s, b1s, w2s, b2s = W1a, b1a, W2a, b2a
        in_maps.append({
            "xs": x[BPC * c:BPC * (c + 1)],
            "w1": bf(pad_w(w1s, 1024, D1P)),
            "b1": pad_b(b1s, D1P),
            "w2": bf(pad_w(w2s, D1P, D2P)),
            "b2": pad_b(b2s, D2P),
            "jwa": bf(jwa),
            "jwa4": bf(jwa4),
            "jwb": bf(jwb),
            "identb": bf(identb),
            "diagm": bf(diagm),
            "posm": posm,
        })
    return in_maps


def kernel(x, W1c, b1c, W2c, b2c, W1a, b1a, W2a, b2a):
    global LAST_RESULT
    trace = bool(os.environ.get("BASS_TRACE"))
    if trace:
        _install_ntff_hook()
    from concourse import bass_utils
    if trace:
        bass_utils.upload_artifacts = lambda tmpdir: "local://skipped"

    if "nc" not in _CACHE:
        _CACHE["nc"] = _build_nc()
    nc = _CACHE["nc"]

    in_maps = _host_inputs(x, W1c, b1c, W2c, b2c, W1a, b1a, W2a, b2a)
    kwargs = {}
    if trace:
        kwargs = {"trace": True, "trace_cores": TRACE_CORES}
    res = bass_utils.run_bass_kernel_spmd(
        nc, in_maps, list(range(NCORES)), **kwargs)
    LAST_RESULT = res
    lout = np.concatenate(
        [np.asarray(res.results[c]["lout"], np.float64) for c in range(NCORES)])
    nll = np.log(lout[:, 0]) - lout[:, 1]
    return np.asarray(nll.mean(), dtype=np.float32)


# revision 19
# speedup vs baseline: 1.6665x; 1.4396x over previous
"""Trainium2 Bass kernel for the ESM contrastive projection head loss.

Problem (hardcoded): x [512, 512, 960] f32; two 2-layer MLPs (codon for batch
rows 0:256, amino for 256:512) applied to mean-pooled x; pairwise cosine
similarity of the concatenated projections z [512, 240]; diag-masked,
temperature-scaled InfoNCE-style NLL, mean over rows.

Strategy: data-parallel over batch across 8 NeuronCores (64 rows each).
x is cast to fp8 e3m4 on the host (4 mantissa bits; |x|<6 well inside the
+-15.5 range; pooled-mean quantization error ~1e-4 of the loss), quartering
the HBM bytes each core streams to 31.5 MB.  The stream runs on the gpsimd
(SWDGE) ring as cast DMAs e3m4->bf16; every slab is two DMAs, the second
with a CCE accumulate-add, so the first level of the plane-reduction tree
happens inline in the DMA (CCE adds in fp32, e3m4->bf16 cast is exact, so
this is numerically identical to the DVE add it replaces).  DVE reduces the
remaining 8->1 planes per partition and windowed bf16 matmuls accumulate the
transposed pooled tile pT [960, 64] in PSUM (no transposes needed before the
MLP).  The MLP runs in bf16 (weights zero-padded to 1024/512/256 so every
contraction chunk is 128 wide).  Projections are normalized locally (scaled
by sqrt(10)/|z| so the 1/T factor is pre-baked) and gathered as bf16.

Tail: the batch half is processed as two 32-row blocks.  Block A (rows
0:32) completes its pooled columns ~55% into the stream, so its MLP, norm
and 16 KB allgather (~20 us of mesh latency) all hide under the remaining
stream; only block B's MLP+norm+allgather and the 64x512 logits/exp remain
after the last DMA byte.  The logits are z_hat^T z_hat plus a rank-64
diag-mask matmul accumulated in PSUM; sum-of-exp straight out of PSUM; the
final log and mean happen on the host.
"""
import contextlib
import ctypes
import os
import sys
import types

import numpy as np

B = 512
S = 512
D = 960
NCORES = 8
BPC = B // NCORES           # 64 batch rows per core
INV_T = 10.0
EPS = 1e-8
D1P = 512                   # padded hidden (480 real)
D2P = 256                   # padded out (240 real)
DKC = 8                     # contraction chunks over D (7x128 + 64)
NEG_BIG = -1.0e6

N4SLAB = 14                 # 4-row slabs covering rows 0..55
HBLK = BPC // 2             # 32-row tail blocks

DEBUG_ZN = bool(int(os.environ.get("BASS_DEBUG_ZN", "0")))
WARMUP_CC = bool(int(os.environ.get("BASS_WARMUP_CC", "1")))

_CACHE = {}
LAST_RESULT = None
TRACE_CORES = [int(c) for c in os.environ.get("BASS_TRACE_CORES", "0").split(",")]


def _install_ntff_hook():
    """Make run_bass_kernel_spmd(trace=True) work under axon (test.py only)."""
    if "antenv.axon_hooks" in sys.modules:
        return
    so_path = "/opt/axon/libaxon_pjrt.so"
    try:
        lib = ctypes.CDLL(so_path)
    except OSError:
        return
    if not hasattr(lib, "axon_start_nrt_profile"):
        return
    lib.axon_start_nrt_profile.argtypes = [ctypes.POINTER(ctypes.c_int64), ctypes.c_size_t]
    lib.axon_start_nrt_profile.restype = ctypes.c_int64
    lib.axon_stop_nrt_profile.argtypes = [ctypes.c_char_p]
    lib.axon_stop_nrt_profile.restype = ctypes.c_int64

    @contextlib.contextmanager
    def _hook(output_dir, device_ids):
        import jax
        jax.devices()
        if device_ids:
            ids = (ctypes.c_int64 * len(device_ids))(*device_ids)
            rc = lib.axon_start_nrt_profile(ids, len(device_ids))
        else:
            rc = lib.axon_start_nrt_profile(None, 0)
        if rc != 0:
            raise RuntimeError(f"axon_start_nrt_profile rc={rc}")
        try:
            yield
        finally:
            n = lib.axon_stop_nrt_profile(str(output_dir).encode())
            print(f"profile: {n} file(s) written to {output_dir}", file=sys.stderr)

    mod = types.ModuleType("antenv.axon_hooks")
    mod.get_axon_ntff_profile_hook = lambda: _hook
    mod.set_axon_ntff_profile_hook = lambda h: None
    sys.modules["antenv.axon_hooks"] = mod


def _build_nc():
    import concourse.tile as tile
    from concourse import bacc, mybir

    f32 = mybir.dt.float32
    bf16 = mybir.dt.bfloat16
    f8 = mybir.dt.float8e4
    add = mybir.AluOpType.add
    mult = mybir.AluOpType.mult
    amax = mybir.AluOpType.max
    AF = mybir.ActivationFunctionType

    nc = bacc.Bacc("TRN2", target_bir_lowering=False, debug=False,
                   enable_asserts=False, num_devices=NCORES)

    xs = nc.dram_tensor("xs", [BPC, S, D], f8, kind="ExternalInput").ap()
    w1 = nc.dram_tensor("w1", [1024, D1P], bf16, kind="ExternalInput").ap()
    b1 = nc.dram_tensor("b1", [D1P], f32, kind="ExternalInput").ap()
    w2 = nc.dram_tensor("w2", [D1P, D2P], bf16, kind="ExternalInput").ap()
    b2 = nc.dram_tensor("b2", [D2P], f32, kind="ExternalInput").ap()
    jwa = nc.dram_tensor("jwa", [128, 128], bf16, kind="ExternalInput").ap()
    jwa4 = nc.dram_tensor("jwa4", [128, 128], bf16, kind="ExternalInput").ap()
    jwb = nc.dram_tensor("jwb", [128, 128], bf16, kind="ExternalInput").ap()
    identb = nc.dram_tensor("identb", [BPC, BPC], bf16, kind="ExternalInput").ap()
    diagm = nc.dram_tensor("diagm", [BPC, B], bf16, kind="ExternalInput").ap()
    posm = nc.dram_tensor("posm", [BPC, B], f32, kind="ExternalInput").ap()
    out = nc.dram_tensor("lout", [BPC, 2], f32, kind="ExternalOutput").ap()
    dbg_zn = (nc.dram_tensor("dbg_zn", [128, 2, BPC], f32,
                             kind="ExternalOutput").ap() if DEBUG_ZN else None)
    dbg_zft = (nc.dram_tensor("dbg_zft", [128, 2, B], f32,
                              kind="ExternalOutput").ap() if DEBUG_ZN else None)

    RINGS = None  # set inside the tile context

    with tile.TileContext(nc) as tc:
        with contextlib.ExitStack() as ctx:
            RINGS = (nc.scalar, nc.sync, nc.gpsimd)
            ep = ctx.enter_context
            consts = ep(tc.tile_pool(name="consts", bufs=1))
            xpool = ep(tc.tile_pool(name="xslab", bufs=3))
            apool = ep(tc.tile_pool(name="acc", bufs=6))
            spool = ep(tc.tile_pool(name="small", bufs=1))
            scr = ep(tc.tile_pool(name="scratch", bufs=1))
            dram = ep(tc.tile_pool(name="dram", bufs=1, space="DRAM"))
            ppool = ep(tc.tile_pool(name="ppool", bufs=1, space="PSUM"))
            psmm = ep(tc.tile_pool(name="psmm", bufs=2, space="PSUM"))
            ps1 = ep(tc.tile_pool(name="ps1", bufs=1, space="PSUM"))

            # --- window matrices first on the ACT ring (tiny; needed by the
            # first pool matmuls a few us in) ---
            jwa_sb = consts.tile([128, 128], bf16, tag="jwa")
            nc.scalar.dma_start(jwa_sb[:], jwa)
            jwa4_sb = consts.tile([128, 128], bf16, tag="jwa4")
            nc.scalar.dma_start(jwa4_sb[:], jwa4)
            jwb_sb = consts.tile([128, 128], bf16, tag="jwb")
            nc.scalar.dma_start(jwb_sb[:], jwb)

            if WARMUP_CC:
                # warm up the collective path early (junk payload straight
                # from DRAM so no compute dependency delays the trigger; the
                # readback, which waits on the collective, goes at the END of
                # the SP ring consts so it cannot stall anything)
                wb = dram.tile([BPC, 8], bf16, tag="wb")
                wg = dram.tile([B, 8], bf16, tag="wg")
                nc.gpsimd.dma_start(wb[:], jwa[0:BPC, 0:8])
                nc.gpsimd.collective_compute(
                    "AllGather", mybir.AluOpType.bypass,
                    replica_groups=[list(range(NCORES))],
                    ins=[wb.opt()], outs=[wg.opt()],
                )

            # Sacrificial PE warm-up: the first-ever matmuls after PE idle
            # come out corrupted, so burn them on a throwaway piece into a
            # separate PSUM region (own start/stop group), folded into the
            # output with weight 0 so it is not dead code.
            # Multi-plane so it absorbs the first-large-DMA truncation on the
            # gpsimd ring (its first DMA with a big middle dim only delivers
            # the first two planes -- the x stream follows it); the in-place
            # add likewise burns the first DVE op.
            sac = xpool.tile([128, 4, D], bf16, tag="slab1", bufs=2)
            nc.gpsimd.dma_start(
                sac[:], xs[0:1].rearrange("b (q m) d -> (b q) m d", m=4))
            nc.vector.tensor_tensor(sac[:, 0:2, :], sac[:, 0:2, :],
                                    sac[:, 2:4, :], add)
            sac_ps = psmm.tile([128, BPC], f32, tag="sacp", bufs=1)
            for k in range(DKC):
                cw = 128 if k < 7 else 64
                nc.tensor.matmul(sac_ps[0:cw, :],
                                 sac[:, 0, 128 * k:128 * k + cw],
                                 jwb_sb[:, 0:64], start=(k == 0), stop=(k == DKC - 1))
            sacv = spool.tile([1, 1], f32, tag="sacv")
            nc.vector.tensor_copy(sacv[:], sac_ps[0:1, 0:1])

            # --- remaining constants on the SP ring (idle until the tail) ---
            w1_sb = consts.tile([128, DKC, D1P], bf16, tag="w1")
            nc.sync.dma_start(w1_sb[:], w1.rearrange("(k p) j -> p k j", p=128))
            w2_sb = consts.tile([128, 4, D2P], bf16, tag="w2")
            nc.sync.dma_start(w2_sb[:], w2.rearrange("(k p) j -> p k j", p=128))
            b1_sb = consts.tile([128, 4], f32, tag="b1")
            nc.sync.dma_start(b1_sb[:], b1.rearrange("(g p) -> p g", p=128))
            b2_sb = consts.tile([128, 2], f32, tag="b2")
            nc.sync.dma_start(b2_sb[:], b2.rearrange("(g p) -> p g", p=128))
            identb_sb = consts.tile([BPC, BPC], bf16, tag="identb")
            nc.sync.dma_start(identb_sb[:], identb)
            diagm_sb = consts.tile([BPC, B], bf16, tag="diagm")
            nc.sync.dma_start(diagm_sb[:], diagm)
            posm_sb = consts.tile([BPC, B], f32, tag="posm")
            nc.sync.dma_start(posm_sb[:], posm)
            if WARMUP_CC:
                wg_sb = spool.tile([BPC, 1], bf16, tag="wg")
                nc.sync.dma_start(wg_sb[:], wg[0:BPC, 0:1])

            ones_sb = consts.tile([128, 1], f32, tag="ones")
            nc.vector.memset(ones_sb[:], 1.0)
            onesb_sb = consts.tile([1, 128], bf16, tag="onesb")
            nc.vector.memset(onesb_sb[:], 1.0)
            zeros_sb = consts.tile([128, BPC], f32, tag="zeros")
            nc.vector.memset(zeros_sb[:], 0.0)

            # --- phase A: stream x, accumulate pooled^T in PSUM as two
            # 32-column blocks (rows 0:32 / 32:64).  jw windows carry 1/512
            # so the matmuls emit the mean. ---
            # one full PSUM bank per block (2 KB): bank-sharing two
            # accumulation groups corrupts the group-opening slab's columns
            pT_ps = [ppool.tile([128, DKC, BPC], f32, tag="pTA", name="pTA"),
                     ppool.tile([128, DKC, BPC], f32, tag="pTB", name="pTB")]

            def pool_mms(acc_ap, jw, blk, start, stop):
                for k in range(DKC):
                    cw = 128 if k < 7 else 64
                    nc.tensor.matmul(pT_ps[blk][0:cw, k, 0:HBLK],
                                     acc_ap[:, 128 * k:128 * k + cw],
                                     jw, start=start, stop=stop)

            # per-block tail: pT -> MLP -> norm -> zn block -> gather
            zn_sb = spool.tile([128, 2, BPC], bf16, tag="zn")
            zfT_sb = spool.tile([128, 2, B], bf16, tag="zfT")
            zgs = []

            def tail_block(blk):
                c0 = HBLK * blk
                pT_sb = spool.tile([128, DKC, HBLK], bf16, tag=f"pTsb{blk}")
                nc.vector.tensor_copy(pT_sb[:, 0:7, :],
                                     pT_ps[blk][:, 0:7, 0:HBLK])
                nc.vector.tensor_copy(pT_sb[0:64, 7, :],
                                      pT_ps[blk][0:64, 7, 0:HBLK])
                # MLP layer 1: h^T [512(pad), 32] = relu(W1^T pT + b1)
                h_sb = spool.tile([128, 4, HBLK], bf16, tag=f"h{blk}")
                for jg in range(4):
                    hp = psmm.tile([128, HBLK], f32, tag="mm")
                    for k in range(DKC):
                        cw = 128 if k < 7 else 64
                        nc.tensor.matmul(hp[:],
                                         w1_sb[0:cw, k, 128 * jg:128 * (jg + 1)],
                                         pT_sb[0:cw, k, :],
                                         start=(k == 0), stop=(k == 7))
                    nc.vector.scalar_tensor_tensor(h_sb[:, jg, :], hp[:],
                                                   b1_sb[:, jg:jg + 1],
                                                   zeros_sb[:, 0:HBLK],
                                                   add, amax)
                # MLP layer 2: z^T [256(pad), 32] = W2^T h^T + b2
                zT_sb = spool.tile([128, 2, HBLK], f32, tag=f"zT{blk}")
                for og in range(2):
                    zp = psmm.tile([128, HBLK], f32, tag="mm")
                    for k in range(4):
                        nc.tensor.matmul(zp[:],
                                         w2_sb[:, k, 128 * og:128 * (og + 1)],
                                         h_sb[:, k, :], start=(k == 0), stop=(k == 3))
                    nc.vector.tensor_scalar_add(zT_sb[:, og, :], zp[:],
                                                b2_sb[:, og:og + 1])
                # normalize locally: zn = z * sqrt(10)/|z| (1/T pre-baked)
                lsq = scr.tile([128, 2, HBLK], f32, tag=f"lsq{blk}")
                nc.vector.tensor_tensor(lsq[:], zT_sb[:], zT_sb[:], mult)
                nlq_ps = psmm.tile([1, HBLK], f32, tag="nlq", bufs=1)
                nc.tensor.matmul(nlq_ps[:], ones_sb[:], lsq[:, 0, :],
                                 start=True, stop=False)
                nc.tensor.matmul(nlq_ps[:], ones_sb[:], lsq[:, 1, :],
                                 start=False, stop=True)
                nlr_sb = spool.tile([1, HBLK], f32, tag=f"nlr{blk}")
                nc.scalar.activation(nlr_sb[:], nlq_ps[:], AF.Sqrt, scale=0.1)
                inv_sb = spool.tile([1, HBLK], f32, tag=f"inv{blk}")
                nc.vector.reciprocal(inv_sb[:], nlr_sb[:])
                invb_sb = spool.tile([1, HBLK], bf16, tag=f"invb{blk}")
                nc.vector.tensor_copy(invb_sb[:], inv_sb[:])
                invp = psmm.tile([128, HBLK], f32, tag="invp", bufs=1)
                nc.tensor.matmul(invp[:], onesb_sb[:], invb_sb[:],
                                 start=True, stop=True)
                for og in range(2):
                    nc.vector.tensor_tensor(zn_sb[:, og, c0:c0 + HBLK],
                                            zT_sb[:, og, :], invp[:], mult)

            def gather_block(blk):
                c0 = HBLK * blk
                zb = dram.tile([2 * 128, HBLK], bf16, tag=f"zb{blk}")
                zg = dram.tile([2 * 128 * NCORES, HBLK], bf16, tag=f"zg{blk}")
                zgs.append(zg)
                nc.sync.dma_start(
                    zb[:].rearrange("(og p) b -> p og b", p=128),
                    zn_sb[:, :, c0:c0 + HBLK])
                nc.gpsimd.collective_compute(
                    "AllGather", mybir.AluOpType.bypass,
                    replica_groups=[list(range(NCORES))],
                    ins=[zb.opt()], outs=[zg.opt()],
                )

            def load_block(blk):
                c0 = HBLK * blk
                zgv = zgs[blk][:].rearrange("(c r) b -> r c b", r=256)
                for og in range(2):
                    dst = zfT_sb[:, og, :].rearrange(
                        "p (c b) -> p c b", b=BPC)[:, :, c0:c0 + HBLK]
                    nc.sync.dma_start(dst, zgv[128 * og:128 * (og + 1)])

            def stream_slab4(i):
                r0 = 4 * i
                blk = 0 if r0 < HBLK else 1
                co = 64 - r0 if blk == 0 else 96 - r0
                jw = jwa4_sb[:, co:co + HBLK]
                t = xpool.tile([128, 8, D], bf16, tag="slab", bufs=3)
                t2 = xpool.tile([128, 8, D], bf16, tag="slabb", bufs=3)
                v = xs[r0:r0 + 4].rearrange("b (q h m) d -> h (b q) m d",
                                            h=2, m=8)
                nc.gpsimd.dma_start(t[:], v[0])
                nc.gpsimd.dma_start(t2[:], v[1])
                nc.vector.tensor_tensor(t[:], t[:], t2[:], add)
                nc.vector.tensor_tensor(t[:, 0:4, :], t[:, 0:4, :],
                                        t[:, 4:8, :], add)
                nc.vector.tensor_tensor(t[:, 0:2, :], t[:, 0:2, :],
                                        t[:, 2:4, :], add)
                acc = apool.tile([128, D], bf16, tag="acc")
                nc.vector.tensor_tensor(acc[:], t[:, 0, :], t[:, 1, :], add)
                pool_mms(acc, jw, blk, start=(i in (0, HBLK // 4)),
                         stop=(i == 7))

            # rows 0:32 (block A) as eight 4-row slabs (two cast DMAs each)
            for i in range(0, 8):
                stream_slab4(i)

            # block A tail computes while rows 32:64 stream
            tail_block(0)

            for i in range(8, N4SLAB):
                stream_slab4(i)

            # rows 56:62 as three 2-row slabs + row 62
            for n_, r0 in enumerate((56, 58, 60)):
                t = xpool.tile([128, 4, D], bf16, tag="slab2", bufs=2,
                               name=f"t{r0}")
                t2 = xpool.tile([128, 4, D], bf16, tag="slab2b", bufs=2,
                                name=f"u{r0}")
                v = xs[r0:r0 + 2].rearrange("b (q h m) d -> h (b q) m d",
                                            h=2, m=4)
                nc.gpsimd.dma_start(t[:], v[0])
                nc.gpsimd.dma_start(t2[:], v[1])
                nc.vector.tensor_tensor(t[:], t[:], t2[:], add)
                nc.vector.tensor_tensor(t[:, 0:2, :], t[:, 0:2, :],
                                        t[:, 2:4, :], add)
                acc = apool.tile([128, D], bf16, tag="acc")
                nc.vector.tensor_tensor(acc[:], t[:, 0, :], t[:, 1, :], add)
                pool_mms(acc, jwa_sb[:, 96 - r0:128 - r0], 1,
                         start=False, stop=False)

            # row 62
            t62 = xpool.tile([128, 2, D], bf16, tag="slab62", bufs=1)
            t62b = xpool.tile([128, 2, D], bf16, tag="slab62b", bufs=1)
            v62 = xs[62:63].rearrange("b (q h m) d -> h (b q) m d", h=2, m=2)
            nc.gpsimd.dma_start(t62[:], v62[0])
            nc.gpsimd.dma_start(t62b[:], v62[1])
            acc = apool.tile([128, D], bf16, tag="acc")
            nc.vector.tensor_tensor(t62[:], t62[:], t62b[:], add)
            nc.vector.tensor_tensor(acc[:], t62[:, 0, :], t62[:, 1, :], add)
            pool_mms(acc, jwb_sb[:, 34:66], 1, start=False, stop=False)

            # block A gather fires mid-stream
            gather_block(0)
            load_block(0)

            # row 63 as four [128, 960] quarter pieces: no DVE reduce at all,
            # so the post-stream critical path is just 8 small matmuls.
            for q in range(4):
                piece = xpool.tile([128, D], bf16, tag="piece", bufs=4)
                src = xs[63:64, 128 * q:128 * (q + 1), :].rearrange(
                    "b s d -> (b s) d")
                nc.gpsimd.dma_start(piece[:], src)
                pool_mms(piece, jwb_sb[:, 33:65], 1, start=False, stop=(q == 3))

            # --- block B tail + gather (the only exposed tail work) ---
            tail_block(1)
            gather_block(1)
            load_block(1)
            if DEBUG_ZN:
                zn_dbg = scr.tile([128, 2, BPC], f32, tag="zndbg")
                nc.vector.tensor_copy(zn_dbg[:], zn_sb[:])
                nc.sync.dma_start(dbg_zn, zn_dbg[:])
                zfT_dbg = scr.tile([128, 2, B], f32, tag="zftdbg")
                nc.vector.tensor_copy(zfT_dbg[:], zfT_sb[:])
                nc.sync.dma_start(dbg_zft, zfT_dbg[:])

            # --- logits [64, 512] = zn^T zfT (=10*cos) + diag mask, in PSUM.
            # The diag-mask matmul goes first so it runs during the gather. ---
            s_ps = ps1.tile([BPC, B], f32, tag="sp")
            nc.tensor.matmul(s_ps[:], identb_sb[:], diagm_sb[:],
                             start=True, stop=False)
            nc.tensor.matmul(s_ps[:], zn_sb[:, 0, :], zfT_sb[:, 0, :],
                             start=False, stop=False)
            nc.tensor.matmul(s_ps[:], zn_sb[:, 1, :], zfT_sb[:, 1, :],
                             start=False, stop=True)

            # --- sum of exp(logits) straight out of PSUM (logits <= 10 so no
            # max-shift is needed); the final ln happens on the host.  The
            # pos-extract (DVE) and the exp (ACT) write different dummies so
            # they run concurrently. ---
            res_sb = spool.tile([BPC, 2], f32, tag="res")
            e_sb = scr.tile([BPC, B], f32, tag="esb")
            e2_sb = scr.tile([BPC, B], f32, tag="esb2")
            nc.vector.scalar_tensor_tensor(e2_sb[:], s_ps[:], 1.0, posm_sb[:],
                                           mult, mult,
                                           accum_out=res_sb[:, 1:2])
            esum = spool.tile([BPC, 1], f32, tag="esum")
            nc.scalar.activation(e_sb[:], s_ps[:], AF.Exp, accum_out=esum[:])
            if WARMUP_CC:
                # keep the warmup collective alive: esum += 0 * wg
                nc.vector.scalar_tensor_tensor(res_sb[:, 0:1], wg_sb[:], 0.0,
                                               esum[:], mult, add)
            else:
                nc.vector.tensor_copy(res_sb[:, 0:1], esum[:])
            # keep the PE warm-up matmuls alive: res[0,0] += 0 * sacv
            nc.vector.scalar_tensor_tensor(res_sb[0:1, 0:1], sacv[:], 0.0,
                                           res_sb[0:1, 0:1], mult, add)

            nc.sync.dma_start(out, res_sb[:])

    nc.compile()
    return nc


def _host_inputs(x, W1c, b1c, W2c, b2c, W1a, b1a, W2a, b2a):
    import ml_dtypes
    # RNE cast to fp8 e4m3 on the host with a x16 prescale (pushes the
    # data out of the subnormal range, which some HW paths flush to zero;
    # 16*|x| < 96, well inside the +-240 range; the jw pooling windows carry
    # 1/(512*16) to undo the scale).  Quarters the streamed HBM bytes.
    x = (np.ascontiguousarray(np.asarray(x, dtype=np.float32)) * 16.0).astype(
        ml_dtypes.float8_e4m3)
    # jwa: 2-row slab i selects columns [64-2i, 128-2i); partition p (batch
    # half p//64) must hit output row 2i + p//64, so the fixed column is
    # 64 + p//64.  jwb: every partition hits the single row r via column 64.
    # Values hold the mean's 1/512 (exact in bf16).
    jwa = np.zeros((128, 128), dtype=np.float32)
    jwa[np.arange(128), 64 + np.arange(128) // 64] = 1.0 / (S * 16)
    jwa4 = np.zeros((128, 128), dtype=np.float32)
    jwa4[np.arange(128), 64 + np.arange(128) // 32] = 1.0 / (S * 16)
    jwb = np.zeros((128, 128), dtype=np.float32)
    jwb[:, 64] = 1.0 / (S * 16)
    identb = np.eye(BPC, dtype=np.float32)

    def bf(a):
        import ml_dtypes
        return np.asarray(a, ml_dtypes.bfloat16)

    def pad_w(w, rows, cols):
        wp = np.zeros((rows, cols), dtype=np.float32)
        wi = np.asarray(w, np.float32)
        wp[:wi.shape[0], :wi.shape[1]] = wi
        return wp

    def pad_b(b, n):
        bp = np.zeros((n,), dtype=np.float32)
        bi = np.asarray(b, np.float32)
        bp[:bi.shape[0]] = bi
        return bp

    in_maps = []
    for c in range(NCORES):
        rows = np.arange(BPC)
        gl = BPC * c + rows
        diagm = np.zeros((BPC, B), dtype=np.float32)
        diagm[rows, gl] = NEG_BIG
        posm = np.zeros((BPC, B), dtype=np.float32)
        posm[rows, (gl + B // 2) % B] = 1.0
        if c < NCORES // 2:
            w1s, b1s, w2s, b2s = W1c, b1c, W2c, b2c
        else:
            w1s, b1s, w2s, b2s = W1a, b1a, W2a, b2a
        in_maps.append({
            "xs": x[BPC * c:BPC * (c + 1)],
            "w1": bf(pad_w(w1s, 1024, D1P)),
            "b1": pad_b(b1s, D1P),
            "w2": bf(pad_w(w2s, D1P, D2P)),
            "b2": pad_b(b2s, D2P),
            "jwa": bf(jwa),
            "jwa4": bf(jwa4),
            "jwb": bf(jwb),
            "identb": bf(identb),
            "diagm": bf(diagm),
            "posm": posm,
        })
    return in_maps


def kernel(x, W1c, b1c, W2c, b2c, W1a, b1a, W2a, b2a):
    global LAST_RESULT
    trace = bool(os.environ.get("BASS_TRACE"))
    if trace:
        _install_ntff_hook()
    from concourse import bass_utils
    if trace:
        bass_utils.upload_artifacts = lambda tmpdir: "local://skipped"

    if "nc" not in _CACHE:
        _CACHE["nc"] = _build_nc()
    nc = _CACHE["nc"]

    in_maps = _host_inputs(x, W1c, b1c, W2c, b2c, W1a, b1a, W2a, b2a)
    kwargs = {}
    if trace:
        kwargs = {"trace": True, "trace_cores": TRACE_CORES}
    res = bass_utils.run_bass_kernel_spmd(
        nc, in_maps, list(range(NCORES)), **kwargs)
    LAST_RESULT = res
    lout = np.concatenate(
        [np.asarray(res.results[c]["lout"], np.float64) for c in range(NCORES)])
    nll = np.log(lout[:, 0]) - lout[:, 1]
    return np.asarray(nll.mean(), dtype=np.float32)
